# revision 17
# baseline (speedup 1.0000x reference)
"""Trainium2 Bass kernel for nn_BlockwiseHadamardInputWrapper.

Computes out = (blockwise-Hadamard-128 of x along last dim) @ W.T + b
for x [2, 4096, 4096] f32, W [4096, 4096] f32, b [4096] f32.

Strategy (8 NeuronCores, data-parallel over the 8192 token rows):
  * The Hadamard is folded into the weights on the host: H is symmetric,
    so (x (I kron H)) W^T = x ((I kron H) W^T). The device then runs a
    single plain GEMM out = x @ Weff + b with
    Weff = blockdiag(H/sqrt(128)) @ W.T, computed once host-side.
  * Mixed precision, tuned empirically against the 2e-2 rel-err gate:
    the first NF8=4 contraction blocks run as fp8e4m3 DoubleRow matmuls
    (2 k-blocks per PE pass, 2 MACs/cell/cycle) with scales folded as
    (x/8)@(W*8) so they accumulate exactly into the same PSUM group as
    the remaining 28 bf16 blocks. Measured rel err: bf16-only 2.4e-3,
    with nf8=4 fp8 blocks 1.5e-2 (gate 2e-2; nf8=8 would fail at 2.0e-2).
  * Outputs evicted as bf16 and upcast on the host. Per-core HBM
    traffic: x 7.5 MiB + Weff 30 MiB + out 8 MiB.
  * Host: flatten x to [8192, 4096], shard 1024 rows per core,
    pre-transpose each shard to xT [4096, 1024] so the contraction dim
    lands on SBUF partitions; quantize per the k-block split above.
  * Device: a PE warmup burst flips the HAM clock gate to 2.4 GHz while
    the first tiles stream in. The GEMM runs 8 out-feature passes; each
    pass holds 8 PSUM accumulators (one per 128-token tile) and streams
    the contraction blocks k-contiguously, so the PE never idles
    (steady-state issue gap = the 216 ns N=512 bf16 floor).
  * DMA routing (measured-critical): x arrives as small independent
    tiles alternating scalar/gpsimd rings so early matmuls wait on
    256 KiB, not megabytes; W streams per-k on sync (pass 0, even k)
    and scalar (odd k, later passes), prefetched one pass ahead and
    emitted before evictions so out DMAs never head-of-line block a
    weight fetch; bias rides gpsimd after the x tiles.
"""

import numpy as np
import ml_dtypes

import concourse.mybir as mybir
import concourse.tile as tile
from concourse import bacc
from concourse.bass_utils import run_bass_kernel_spmd

N_CORES = 8
B, S, D, O = 2, 4096, 4096, 4096
TOK = B * S                # 8192 token rows
TOK_PC = TOK // N_CORES    # 1024 per core
BLOCK = 128
NK = D // BLOCK            # 32 contraction blocks
NM = TOK_PC // 128         # 8 token tiles per core
NCH = 512                  # out-feature chunk (one PSUM bank in f32)
NN = O // NCH              # 8 out-feature chunks
NF8 = 4                    # leading k-blocks in fp8 (DoubleRow pairs)
NP8 = NF8 // 2             # DoubleRow pairs
NKB = NK - NF8             # bf16 k-blocks
A8 = np.float32(8.0)       # fp8 scale: (x/8) @ (W*8)
XCHUNKS = (1, 1, 1, 1, 2, 2, 2, 2, 4, 4, 4, 4)   # bf16 x tile sizes
N_WARMUP = 32              # PE warmup matmuls to flip the HAM gate early

_F32 = mybir.dt.float32
_BF16 = mybir.dt.bfloat16
_FP8 = mybir.dt.float8e4
_BF16_NP = np.dtype(ml_dtypes.bfloat16)
_FP8_NP = np.dtype(ml_dtypes.float8_e4m3)

assert sum(XCHUNKS) == NKB


def _hadamard_norm(n: int) -> np.ndarray:
    """Normalized Sylvester Hadamard matrix H/sqrt(n)."""
    H = np.array([[1.0]], dtype=np.float32)
    while H.shape[0] < n:
        H = np.block([[H, H], [H, -H]])
    return (H / np.sqrt(np.float32(n))).astype(np.float32)


def build_nc():
    nc = bacc.Bacc("TRN2", target_bir_lowering=False, debug=False,
                   num_devices=N_CORES)
    # fp8 x pairs: [pair, p, j, t] = x[(2*pair+j)*128 + p, t] / 8
    x8T = nc.dram_tensor("x8T", [NP8, 128, 2, TOK_PC], _FP8,
                         kind="ExternalInput")
    # bf16 x, k-blocks NF8..NK-1: [(k-NF8)*128 + p, t]
    xT = nc.dram_tensor("xT", [NKB * 128, TOK_PC], _BF16,
                        kind="ExternalInput")
    # fp8 W pairs: [pair, n, p, j, c] = Weff[(2*pair+j)*128+p, n*512+c] * 8
    w8T = nc.dram_tensor("w8T", [NP8, NN, 128, 2, NCH], _FP8,
                         kind="ExternalInput")
    # bf16 W tiles, k-blocks NF8..NK-1
    wTt = nc.dram_tensor("wTt", [NKB, NN, 128, NCH], _BF16,
                         kind="ExternalInput")
    bias = nc.dram_tensor("bias", [128, O], _F32, kind="ExternalInput")
    hmat = nc.dram_tensor("hmat", [BLOCK, BLOCK], _BF16, kind="ExternalInput")
    out = nc.dram_tensor("out", [TOK_PC, O], _BF16, kind="ExternalOutput")

    DR = mybir.MatmulPerfMode.DoubleRow
    x_rings = [nc.scalar, nc.gpsimd]
    with tile.TileContext(nc) as tc:
        with tc.tile_pool(name="const", bufs=1) as const:
            h_sb = const.tile([BLOCK, BLOCK], _BF16)
            nc.sync.dma_start(out=h_sb[:], in_=hmat[:])

            with tc.tile_pool(name="xsb", bufs=1) as xp:
                # fp8 x pairs first (256 KiB each) on the scalar ring.
                x8sb = []
                for pr in range(NP8):
                    t8 = xp.tile([128, 2, TOK_PC], _FP8, name=f"x8_{pr}",
                                 tag=f"x8_{pr}")
                    nc.scalar.dma_start(out=t8[:], in_=x8T[pr])
                    x8sb.append(t8)
                # bf16 x tiles, small ones first, ring round-robin.
                xsb = []        # per bf16 k-block: (tile, idx within tile)
                k0 = 0
                for g, kg in enumerate(XCHUNKS):
                    xt_g = xp.tile([128, kg, TOK_PC], _BF16, name=f"x{g}",
                                   tag=f"x{g}")
                    x_rings[g % 2].dma_start(
                        out=xt_g[:],
                        in_=xT[k0 * 128:(k0 + kg) * 128, :]
                        .rearrange("(g p) t -> p g t", g=kg))
                    for j in range(kg):
                        xsb.append((xt_g, j))
                    k0 += kg

                bias_sb = const.tile([128, O], _F32)
                nc.gpsimd.dma_start(out=bias_sb[:], in_=bias[:])

                with tc.tile_pool(name="psW", bufs=1, space="PSUM") as psw:
                    wps = psw.tile([128, BLOCK], _F32)
                    for _ in range(N_WARMUP):
                        nc.tensor.matmul(
                            wps[:], h_sb[:], h_sb[:],
                            start=True, stop=True, skip_group_check=True)

                with tc.tile_pool(name="wtp", bufs=48) as wtp, \
                     tc.tile_pool(name="psB", bufs=1, space="PSUM") as psb, \
                     tc.tile_pool(name="outp", bufs=8) as outp:
                    wt_tiles = {}

                    def fetch_w(n):
                        # W prefetch for pass n; emitted before pass n-1's
                        # evictions so out DMAs never head-of-line block
                        # weight fetches on the scalar ring.
                        for pr in range(NP8):
                            w8 = wtp.tile([128, 2, NCH], _FP8,
                                          name=f"w8_{n}_{pr}", tag="wt")
                            nc.sync.dma_start(out=w8[:], in_=w8T[pr, n])
                            wt_tiles[(n, "f8", pr)] = w8
                        for k in range(NKB):
                            wt = wtp.tile([128, NCH], _BF16,
                                          name=f"wt{n}_{k}", tag="wt")
                            weng = nc.sync if (n == 0 or k % 2 == 0) \
                                else nc.scalar
                            weng.dma_start(out=wt[:], in_=wTt[k, n])
                            wt_tiles[(n, "bf", k)] = wt

                    fetch_w(0)
                    for n in range(NN):
                        pss = [psb.tile([128, NCH], _F32, name=f"psB{n}_{m}",
                                        tag=f"psB{m}") for m in range(NM)]
                        for pr in range(NP8):
                            w8_t = wt_tiles.pop((n, "f8", pr))
                            for m in range(NM):
                                nc.tensor.matmul(
                                    pss[m][:],
                                    x8sb[pr][:, :, m * 128:(m + 1) * 128],
                                    w8_t[:],
                                    start=(pr == 0), stop=False,
                                    perf_mode=DR,
                                    skip_group_check=True)
                        for k in range(NKB):
                            wt_t = wt_tiles.pop((n, "bf", k))
                            xt_g, j = xsb[k]
                            for m in range(NM):
                                nc.tensor.matmul(
                                    pss[m][:],
                                    xt_g[:, j, m * 128:(m + 1) * 128],
                                    wt_t[:],
                                    start=False, stop=(k == NKB - 1),
                                    skip_group_check=True)
                        if n + 1 < NN:
                            fetch_w(n + 1)
                        for m in range(NM):
                            ot = outp.tile([128, NCH], _BF16,
                                           name=f"ot{n}_{m}", tag="ot")
                            nc.vector.tensor_add(
                                ot[:], pss[m][:],
                                bias_sb[:, n * NCH:(n + 1) * NCH])
                            eng = nc.gpsimd if m % 2 == 0 else nc.scalar
                            eng.dma_start(
                                out=out[m * 128:(m + 1) * 128,
                                        n * NCH:(n + 1) * NCH],
                                in_=ot[:])
    nc.compile()
    return nc


_NC_CACHE = None


def _get_nc():
    global _NC_CACHE
    if _NC_CACHE is None:
        _NC_CACHE = build_nc()
    return _NC_CACHE


def make_in_maps(x: np.ndarray, W: np.ndarray, b: np.ndarray):
    xf = x.reshape(TOK, D).astype(np.float32, copy=False)
    # Fold the blockwise Hadamard (incl. its 1/sqrt(128)) into W:
    # Weff = blockdiag(Hn) @ W.T, shape [D, O].
    Hn = _hadamard_norm(BLOCK)
    WT = np.ascontiguousarray(W.astype(np.float32, copy=False).T)
    Weff = np.matmul(Hn[None, :, :], WT.reshape(NK, BLOCK, O))  # [NK,128,O]
    # fp8 W pairs [NP8, NN, 128, 2, NCH]
    w8 = (Weff[:NF8] * A8).reshape(NP8, 2, 128, NN, NCH)
    w8T = np.ascontiguousarray(w8.transpose(0, 3, 2, 1, 4)).astype(_FP8_NP)
    # bf16 W tiles [NKB, NN, 128, NCH]
    wTt = np.ascontiguousarray(
        Weff[NF8:].reshape(NKB, 128, NN, NCH).transpose(0, 2, 1, 3)
    ).astype(_BF16_NP)
    bias_rep = np.ascontiguousarray(
        np.broadcast_to(b.astype(np.float32, copy=False)[None, :], (128, O)))
    hmat = np.ascontiguousarray(
        (_hadamard_norm(BLOCK) * np.sqrt(np.float32(BLOCK)))).astype(_BF16_NP)
    in_maps = []
    for c in range(N_CORES):
        xTc = np.ascontiguousarray(
            xf[c * TOK_PC:(c + 1) * TOK_PC, :].T)      # [D, TOK_PC] f32
        xk = xTc.reshape(NK, BLOCK, TOK_PC)
        x8 = np.ascontiguousarray(
            (xk[:NF8] / A8).reshape(NP8, 2, 128, TOK_PC)
            .transpose(0, 2, 1, 3)).astype(_FP8_NP)
        xb = np.ascontiguousarray(
            xk[NF8:].reshape(NKB * 128, TOK_PC)).astype(_BF16_NP)
        in_maps.append({"x8T": x8, "xT": xb, "w8T": w8T, "wTt": wTt,
                        "bias": bias_rep, "hmat": hmat})
    return in_maps


def run(x, W, b, trace=False):
    nc = _get_nc()
    in_maps = make_in_maps(x, W, b)
    last_err = None
    for attempt in range(3):
        try:
            res = run_bass_kernel_spmd(nc, in_maps, list(range(N_CORES)),
                                       trace=trace)
            break
        except Exception as e:  # transient NRT_EXEC_UNIT_UNRECOVERABLE wedge
            last_err = e
            if "UNRECOVERABLE" not in str(e) and "UNAVAILABLE" not in str(e):
                raise
    else:
        raise last_err
    parts = [np.asarray(res.results[c]["out"]).astype(np.float32)
             for c in range(N_CORES)]
    full = np.concatenate(parts, axis=0).reshape(B, S, O)
    return full, res


def kernel(x: np.ndarray, W: np.ndarray, b: np.ndarray) -> np.ndarray:
    out, _ = run(x, W, b, trace=False)
    return out


# revision 18
# speedup vs baseline: 1.1174x; 1.1174x over previous
"""Trainium2 Bass kernel for nn_BlockwiseHadamardInputWrapper.

Computes out = (blockwise-Hadamard-128 of x along last dim) @ W.T + b
for x [2, 4096, 4096] f32, W [4096, 4096] f32, b [4096] f32.

Strategy (8 NeuronCores, data-parallel over the 8192 token rows):
  * The Hadamard is folded into the weights on the host: H is symmetric,
    so (x (I kron H)) W^T = x ((I kron H) W^T). The device then runs a
    single plain GEMM out = x @ Weff + b with
    Weff = blockdiag(H/sqrt(128)) @ W.T, computed once host-side.
  * GEMM operands are bf16 (the 2e-2 rel-err budget dwarfs bf16
    rounding; measured 3.9e-3 end to end), outputs evicted as bf16 and
    upcast on the host. Per-core HBM traffic: x 8 MiB + Weff 32 MiB +
    out 8 MiB. (fp8 DoubleRow for part of the contraction was tried and
    is numerically fine at 4/32 blocks, but any kernel containing
    DoubleRow matmuls runs the whole PE at 2.0 GHz instead of 2.4 —
    a net loss.)
  * Host: flatten x to [8192, 4096], shard 1024 rows per core,
    pre-transpose each shard to xT [4096, 1024] bf16 so the contraction
    dim lands on SBUF partitions. Weff is tiled [NK, NN, 128, 512] so
    every streamed weight tile is one contiguous 128 KiB read.
  * Device: a PE warmup burst flips the HAM clock gate to 2.4 GHz while
    the first tiles stream in. The GEMM runs 8 out-feature passes; each
    pass holds 8 PSUM accumulators (one per 128-token tile) and streams
    the 32 contraction blocks k-contiguously, so the PE never idles
    (steady-state issue gap = the 216 ns N=512 bf16 floor).
  * DMA routing (measured-critical): x arrives as 13 small tiles
    (1,1,1,1,2,2,2,2,4,4,4,4,4 k-blocks) alternating the scalar and
    gpsimd rings so the first matmul only waits for 256 KiB; W streams
    per-k on sync (pass 0, even k) and scalar (odd k, passes >= 1),
    prefetched one pass ahead and emitted before evictions so out DMAs
    never head-of-line block a weight fetch; bias rides gpsimd after
    the x tiles; outputs alternate gpsimd/scalar.
"""

import numpy as np
import ml_dtypes

import concourse.mybir as mybir
import concourse.tile as tile
from concourse import bacc
from concourse.bass_utils import run_bass_kernel_spmd

N_CORES = 8
B, S, D, O = 2, 4096, 4096, 4096
TOK = B * S                # 8192 token rows
TOK_PC = TOK // N_CORES    # 1024 per core
BLOCK = 128
NK = D // BLOCK            # 32 contraction blocks
NM = TOK_PC // 128         # 8 token tiles per core
NCH = 512                  # out-feature chunk (one PSUM bank in f32)
NN = O // NCH              # 8 out-feature chunks
XCHUNKS = (1, 1, 1, 1, 2, 2, 2, 2, 4, 4, 4, 4, 4)  # k-blocks per x tile
N_WARMUP = 32              # PE warmup matmuls to flip the HAM gate early

_F32 = mybir.dt.float32
_BF16 = mybir.dt.bfloat16
_BF16_NP = np.dtype(ml_dtypes.bfloat16)

assert sum(XCHUNKS) == NK


def _hadamard_norm(n: int) -> np.ndarray:
    """Normalized Sylvester Hadamard matrix H/sqrt(n)."""
    H = np.array([[1.0]], dtype=np.float32)
    while H.shape[0] < n:
        H = np.block([[H, H], [H, -H]])
    return (H / np.sqrt(np.float32(n))).astype(np.float32)


def build_nc():
    nc = bacc.Bacc("TRN2", target_bir_lowering=False, debug=False,
                   num_devices=N_CORES)
    xT = nc.dram_tensor("xT", [D, TOK_PC], _BF16, kind="ExternalInput")
    wTt = nc.dram_tensor("wTt", [NK, NN, 128, NCH], _BF16,
                         kind="ExternalInput")
    bias = nc.dram_tensor("bias", [128, O], _F32, kind="ExternalInput")
    hmat = nc.dram_tensor("hmat", [BLOCK, BLOCK], _BF16, kind="ExternalInput")
    out = nc.dram_tensor("out", [TOK_PC, O], _BF16, kind="ExternalOutput")

    x_rings = [nc.scalar, nc.gpsimd]
    with tile.TileContext(nc) as tc:
        with tc.tile_pool(name="const", bufs=1) as const:
            h_sb = const.tile([BLOCK, BLOCK], _BF16)
            nc.sync.dma_start(out=h_sb[:], in_=hmat[:])

            # x arrives as 13 independent tiles, small ones first, so the
            # first matmul waits for only 256 KiB. Ring round-robin.
            xsb = []        # per k-block: (tile, idx within tile)
            with tc.tile_pool(name="xsb", bufs=1) as xp:
                k0 = 0
                for g, kg in enumerate(XCHUNKS):
                    xt_g = xp.tile([128, kg, TOK_PC], _BF16, name=f"x{g}",
                                   tag=f"x{g}")
                    x_rings[g % 2].dma_start(
                        out=xt_g[:],
                        in_=xT[k0 * 128:(k0 + kg) * 128, :]
                        .rearrange("(g p) t -> p g t", g=kg))
                    for j in range(kg):
                        xsb.append((xt_g, j))
                    k0 += kg

                bias_sb = const.tile([128, O], _F32)
                nc.gpsimd.dma_start(out=bias_sb[:], in_=bias[:])

                with tc.tile_pool(name="psW", bufs=1, space="PSUM") as psw:
                    wps = psw.tile([128, BLOCK], _F32)
                    for _ in range(N_WARMUP):
                        nc.tensor.matmul(
                            wps[:], h_sb[:], h_sb[:],
                            start=True, stop=True, skip_group_check=True)

                with tc.tile_pool(name="wtp", bufs=48) as wtp, \
                     tc.tile_pool(name="psB", bufs=1, space="PSUM") as psb, \
                     tc.tile_pool(name="outp", bufs=8) as outp:
                    wt_tiles = {}

                    def fetch_w(n):
                        # W prefetch for pass n; emitted before pass n-1's
                        # evictions so out DMAs never head-of-line block
                        # weight fetches on the scalar ring.
                        for k in range(NK):
                            wt = wtp.tile([128, NCH], _BF16,
                                          name=f"wt{n}_{k}", tag="wt")
                            weng = nc.sync if (n == 0 or k % 2 == 0) \
                                else nc.scalar
                            weng.dma_start(out=wt[:], in_=wTt[k, n])
                            wt_tiles[(n, k)] = wt

                    fetch_w(0)
                    for n in range(NN):
                        pss = [psb.tile([128, NCH], _F32, name=f"psB{n}_{m}",
                                        tag=f"psB{m}") for m in range(NM)]
                        for k in range(NK):
                            wt_t = wt_tiles.pop((n, k))
                            xt_g, j = xsb[k]
                            for m in range(NM):
                                nc.tensor.matmul(
                                    pss[m][:],
                                    xt_g[:, j, m * 128:(m + 1) * 128],
                                    wt_t[:],
                                    start=(k == 0), stop=(k == NK - 1),
                                    skip_group_check=True)
                        if n + 1 < NN:
                            fetch_w(n + 1)
                        for m in range(NM):
                            ot = outp.tile([128, NCH], _BF16,
                                           name=f"ot{n}_{m}", tag="ot")
                            nc.vector.tensor_add(
                                ot[:], pss[m][:],
                                bias_sb[:, n * NCH:(n + 1) * NCH])
                            eng = nc.gpsimd if m % 2 == 0 else nc.scalar
                            eng.dma_start(
                                out=out[m * 128:(m + 1) * 128,
                                        n * NCH:(n + 1) * NCH],
                                in_=ot[:])
    nc.compile()
    return nc


_NC_CACHE = None


def _get_nc():
    global _NC_CACHE
    if _NC_CACHE is None:
        _NC_CACHE = build_nc()
    return _NC_CACHE


def make_in_maps(x: np.ndarray, W: np.ndarray, b: np.ndarray):
    xf = x.reshape(TOK, D).astype(np.float32, copy=False)
    # Fold the blockwise Hadamard (incl. its 1/sqrt(128)) into W:
    # Weff = blockdiag(Hn) @ W.T, shape [D, O]; tile to [NK, NN, 128, NCH].
    Hn = _hadamard_norm(BLOCK)
    WT = np.ascontiguousarray(W.astype(np.float32, copy=False).T)
    Weff = np.matmul(Hn[None, :, :], WT.reshape(NK, BLOCK, O))
    wTt = np.ascontiguousarray(
        Weff.reshape(NK, 128, NN, NCH).transpose(0, 2, 1, 3)).astype(_BF16_NP)
    bias_rep = np.ascontiguousarray(
        np.broadcast_to(b.astype(np.float32, copy=False)[None, :], (128, O)))
    hmat = np.ascontiguousarray(
        (_hadamard_norm(BLOCK) * np.sqrt(np.float32(BLOCK)))).astype(_BF16_NP)
    in_maps = []
    for c in range(N_CORES):
        xTc = np.ascontiguousarray(
            xf[c * TOK_PC:(c + 1) * TOK_PC, :].T).astype(_BF16_NP)
        in_maps.append(
            {"xT": xTc, "wTt": wTt, "bias": bias_rep, "hmat": hmat})
    return in_maps


def run(x, W, b, trace=False):
    nc = _get_nc()
    in_maps = make_in_maps(x, W, b)
    last_err = None
    for attempt in range(3):
        try:
            res = run_bass_kernel_spmd(nc, in_maps, list(range(N_CORES)),
                                       trace=trace)
            break
        except Exception as e:  # transient NRT_EXEC_UNIT_UNRECOVERABLE wedge
            last_err = e
            if "UNRECOVERABLE" not in str(e) and "UNAVAILABLE" not in str(e):
                raise
    else:
        raise last_err
    parts = [np.asarray(res.results[c]["out"]).astype(np.float32)
             for c in range(N_CORES)]
    full = np.concatenate(parts, axis=0).reshape(B, S, O)
    return full, res


def kernel(x: np.ndarray, W: np.ndarray, b: np.ndarray) -> np.ndarray:
    out, _ = run(x, W, b, trace=False)
    return out
